# revision 1
# baseline (speedup 1.0000x reference)
# Trainium2 Bass kernel for nn_BinaryClassifier (one-hot -> LSTM -> FC).
#
# Data-parallel over batch: 8 sorted sequences per NeuronCore, 8 cores run
# one program on different shards. Per core the LSTM runs 2048 sequential
# steps: each step streams W_hh through the PE as 64 bf16 [128,128]
# stationary tiles against the transposed h state ([128 hidden, 8 batch],
# kept in a 33-slot SBUF ring), with the embedding contribution
# (E = W_ih.T + biases, gathered by token) pre-accumulated into PSUM by an
# identity matmul; embedding columns are produced on the PE 16 steps at a
# time from host-built one-hot moving operands, interleaved into the step
# stream as stall filler. All gate nonlinearities use a single tanh per
# hidden slice (sigmoid(x) = (tanh(x/2)+1)/2 folded into pre-scaled
# weights; h stored as 2h, c as 2c). Whole chunks of h are DMAd to DRAM;
# the host gathers h at t = len-1 and applies the FC during unsharding.
import sys
sys.path.insert(0, '/opt/trn_rl_repo')
from contextlib import ExitStack

import numpy as np
import ml_dtypes

import concourse.bass as bass
import concourse.mybir as mybir
from concourse.tile import TileContext
from concourse.bass import ds
from concourse.bass_utils import run_bass_kernel_spmd

F32 = mybir.dt.float32
BF16 = mybir.dt.bfloat16
AF = mybir.ActivationFunctionType
ALU = mybir.AluOpType

H = 512
V = 25
S = 2048
N_CORES = 8
BLOC = 8          # sequences per core
CH = 16           # steps per embedding chunk
BODY = 2 * CH     # steps per chunk pair
NM = 16           # gate tiles (4H / 128)
NK = 4            # contraction tiles (H / 128)

_TPB_ENGINES = None


def split_multi_waits(nc):
    """walrus in this container supports only ONE sync wait per TPB engine
    instruction; split extra waits onto preceding same-engine NOPs."""
    global _TPB_ENGINES
    if _TPB_ENGINES is None:
        _TPB_ENGINES = {mybir.EngineType.Pool, mybir.EngineType.Activation,
                        mybir.EngineType.PE, mybir.EngineType.DVE,
                        mybir.EngineType.SP}
    ctr = 0
    for fn in nc.m.functions:
        for bb in fn.blocks:
            new = []
            for inst in bb.instructions:
                si = inst.sync_info
                if (si is not None and len(si.on_wait) > 1
                        and inst.engine in _TPB_ENGINES):
                    waits = list(si.on_wait)
                    for w in waits[:-1]:
                        nop = mybir.InstNoOp(name=f"wsplit-{ctr}", ins=[],
                                             outs=[])
                        ctr += 1
                        nop.engine = inst.engine
                        nop.sync_info = mybir.SyncInfo(on_wait=[w],
                                                       on_update=[])
                        new.append(nop)
                    si.on_wait = waits[-1:]
                    inst.sync_info = si
                new.append(inst)
            bb.instructions = new


def _host_prep(tokens, lengths, W_ih, W_hh, b_ih, b_hh, fc_w, fc_b):
    """Full inputs -> list of per-core input dicts (numpy).

    Gate-tile numbering: m = j*4 + g where j = hidden slice (0..3) and
    g in {0:i, 1:f, 2:o, 3:g_cell} (reordered from torch i,f,g,o)."""
    bf = ml_dtypes.bfloat16
    order = np.argsort(-lengths.astype(np.int64), kind='stable')
    toks = np.asarray(tokens)[order]
    lens = np.asarray(lengths)[order].astype(np.int64)

    # rows of W_* are 4H in torch gate order i,f,g,o; our g order: i,f,o,g
    perm = np.concatenate([np.arange(0 * H, 1 * H),      # i
                           np.arange(1 * H, 2 * H),      # f
                           np.arange(3 * H, 4 * H),      # o
                           np.arange(2 * H, 3 * H)])     # g_cell
    # row index of gate-unit (g, j, c) in permuted matrix: g*H + j*128 + c
    Whh_p = np.asarray(W_hh)[perm].astype(np.float32)    # [4H, H]
    E_p = (np.asarray(W_ih) + np.asarray(b_ih)[:, None]
           + np.asarray(b_hh)[:, None])[perm].astype(np.float32)
    # sigmoid(x) = (tanh(x/2)+1)/2: pre-halve i,f,o gate rows so one tanh
    # covers all gates; h is stored as h2 = 2h, so W_hh is halved again.
    ifo = np.zeros(4 * H, bool)
    ifo[0:3 * H] = True                                   # i,f,o rows
    Whh_p[ifo] *= 0.5
    E_p[ifo] *= 0.5
    Whh_p *= 0.5                                          # h2 = 2h convention

    # w_lhsT: [128, NK*NM*128], tile (k, m) at cols (k*NM+m)*128
    # m = j*4+g selects rows g*H + j*128 + (0..127); k selects hidden cols
    w = np.zeros((128, NK * NM * 128), np.float32)
    e = np.zeros((V, NM * 128), np.float32)
    for j in range(4):
        for g in range(4):
            m = j * 4 + g
            rows = slice(g * H + j * 128, g * H + j * 128 + 128)
            for k in range(NK):
                blk = Whh_p[rows, k * 128:(k + 1) * 128]   # [128 rows, 128 k]
                w[:, (k * NM + m) * 128:(k * NM + m + 1) * 128] = blk.T
            e[:, m * 128:(m + 1) * 128] = E_p[rows, :].T   # [V, 128]

    fcw = np.zeros((128, 4), np.float32)
    for j in range(4):
        # hfin holds h2 = 2h, fold the 0.5 into the FC weights
        fcw[:, j] = 0.5 * np.asarray(fc_w)[0, j * 128:(j + 1) * 128]

    per_core = []
    for ci in range(N_CORES):
        bs = slice(ci * BLOC, (ci + 1) * BLOC)
        t_c = toks[bs]                                    # [8, S]
        l_c = lens[bs]                                    # [8]
        oh = np.zeros((V, S * BLOC + 2 * CH * BLOC), np.float32)
        sidx = np.arange(S)
        for b in range(BLOC):
            oh[t_c[b], sidx * BLOC + b] = 1.0
        lcap = np.tile((l_c - 1).astype(np.float32), 4)   # col j*8+b
        lcap = np.broadcast_to(lcap, (128, 32)).copy()
        fcb = np.full((BLOC, 1), np.asarray(fc_b)[0], np.float32)
        per_core.append({
            "ident": np.eye(128, dtype=np.float32).astype(bf),
            "w_lhsT": w.astype(bf),
            "e_lhsT": e.astype(bf),
            "onehot": oh.astype(bf),
            "lcap": lcap,
            "fcw": fcw,
            "fcb": fcb,
        })
    return per_core, order


def _build_nc():
    assert S % BODY == 0
    ITERS = S // BODY
    nc = bass.Bass("TRN2", target_bir_lowering=False, debug=False,
                   num_devices=N_CORES)
    DT = BF16
    w_d = nc.dram_tensor("w_lhsT", [128, NK * NM * 128], DT,
                         kind="ExternalInput").ap()
    e_d = nc.dram_tensor("e_lhsT", [V, NM * 128], DT,
                         kind="ExternalInput").ap()
    oh_d = nc.dram_tensor("onehot", [V, S * BLOC + 2 * CH * BLOC], DT,
                          kind="ExternalInput").ap()
    lcap_d = nc.dram_tensor("lcap", [128, 32], F32, kind="ExternalInput").ap()
    id_d = nc.dram_tensor("ident", [128, 128], DT, kind="ExternalInput").ap()
    fcw_d = nc.dram_tensor("fcw", [128, 4], F32, kind="ExternalInput").ap()
    fcb_d = nc.dram_tensor("fcb", [BLOC, 1], F32, kind="ExternalInput").ap()
    hd_d = nc.dram_tensor("hdump", [128, S * 32], BF16,
                          kind="ExternalOutput").ap()

    with TileContext(nc) as tc, ExitStack() as ctx:
        const = ctx.enter_context(tc.tile_pool(name="const", bufs=1))
        state = ctx.enter_context(tc.tile_pool(name="state", bufs=1))
        scr = ctx.enter_context(tc.tile_pool(name="scr", bufs=6))
        ohp = ctx.enter_context(tc.tile_pool(name="ohp", bufs=2))

        w_sb = const.tile([128, NK * NM * 128], DT, tag="w")
        e_sb = const.tile([V, NM * 128], DT, tag="e")
        lcap = const.tile([128, 32], F32, tag="lcap")
        fcw = const.tile([128, 4], F32, tag="fcw")
        fcb = const.tile([BLOC, 1], F32, tag="fcb")
        ident = const.tile([128, 128], DT, tag="ident")
        nc.sync.dma_start(out=w_sb[:], in_=w_d[:])
        nc.sync.dma_start(out=e_sb[:], in_=e_d[:])
        nc.sync.dma_start(out=lcap[:], in_=lcap_d[:])
        nc.sync.dma_start(out=fcw[:], in_=fcw_d[:])
        nc.sync.dma_start(out=fcb[:], in_=fcb_d[:])
        nc.sync.dma_start(out=ident[:], in_=id_d[:])

        # h ring: body step i reads slot i, writes slot i+1 (33 slots);
        # slot 32 is copied back to slot 0 at body end. Whole chunks of h
        # are DMAd to DRAM so the host can gather h at t = len-1.
        hring = state.tile([128, 33 * 32], DT, tag="hring")
        c_st = state.tile([128, 32], F32, tag="c")
        nc.vector.memset(hring[:, 0:32], 0)
        nc.vector.memset(c_st[:], 0)

        # xg buffers: col sc*128 + m*8 + b  (t-major after permuted copy)
        xgA = state.tile([128, CH * 128], DT, tag="xgA")
        xgB = state.tile([128, CH * 128], DT, tag="xgB")

        with tc.tile_pool(name="psum", bufs=1, space="PSUM") as psum:
            gp = [psum.tile([128, 32], F32, name=f"gp{j}", tag=f"gp{j}")
                  for j in range(4)]
            xgp = psum.tile([128, CH * 128], F32, tag="xgp")

            def prod_mm(oh_tile, col0, m):
                nc.tensor.matmul(
                    xgp[:, m * CH * BLOC:(m + 1) * CH * BLOC],
                    e_sb[:, m * 128:(m + 1) * 128],
                    oh_tile[:, col0:col0 + CH * BLOC],
                    start=(m % 4 == 0), stop=(m % 4 == 3))

            def xg_copy(xg_sb):
                src = xgp[:].rearrange("p (m t b) -> p t m b",
                                       m=NM, t=CH, b=BLOC)
                dst = xg_sb[:].rearrange("p (t m b) -> p t m b",
                                         m=NM, t=CH, b=BLOC)
                nc.scalar.copy(dst, src)

            def produce_xg(oh_tile, col0, xg_sb):
                for m in range(NM):
                    prod_mm(oh_tile, col0, m)
                xg_copy(xg_sb)

            def step(sc, xg_sb, hT, hTn, filler=None):
                # adjacent identity injections (slices 0-2) so ldw-opt can
                # elide the repeated stationary loads; slice 3's stays at
                # its block to avoid a WAR stall on the prior step's tanh
                for j in range(3):
                    nc.tensor.matmul(
                        gp[j][:], ident[:],
                        xg_sb[:, sc * 128 + j * 32:sc * 128 + (j + 1) * 32],
                        start=True, stop=False)
                for j in range(4):
                    if j == 3:
                        nc.tensor.matmul(
                            gp[3][:], ident[:],
                            xg_sb[:, sc * 128 + 96:sc * 128 + 128],
                            start=True, stop=False)
                        if filler is not None:
                            filler()
                    for k in range(NK):
                        for g in range(4):
                            m = j * 4 + g
                            nc.tensor.matmul(
                                gp[j][:, g * 8:(g + 1) * 8],
                                w_sb[:, (k * NM + m) * 128:
                                     (k * NM + m + 1) * 128],
                                hT[:, k * 8:(k + 1) * 8],
                                start=False,
                                stop=(k == NK - 1 and g == 3))
                # two-phase staggered tails: emit slice j's tanh_c/h2 after
                # slice j+1's tanh_all so the last slice's tanh is not
                # queued behind the previous slice's chain on ACT
                tact = [None] * 4
                for j in range(5):
                    if j < 4:
                        tact[j] = scr.tile([128, 32], F32, name=f"ta{j}",
                                           tag=f"tact{j}")
                        nc.scalar.activation(tact[j][:], gp[j][:], AF.Tanh)
                        cs = c_st[:, j * 8:(j + 1) * 8]
                        t1 = scr.tile([128, 8], F32, name=f"t1_{j}",
                                      tag=f"t1_{j}")
                        t2 = scr.tile([128, 8], F32, name=f"t2_{j}",
                                      tag=f"t2_{j}")
                        nc.vector.scalar_tensor_tensor(
                            t1[:], tact[j][:, 0:8], 1.0, tact[j][:, 24:32],
                            op0=ALU.add, op1=ALU.mult)
                        nc.vector.scalar_tensor_tensor(
                            t2[:], tact[j][:, 8:16], 1.0, cs,
                            op0=ALU.add, op1=ALU.mult)
                        nc.vector.scalar_tensor_tensor(
                            cs, t2[:], 0.5, t1[:], op0=ALU.mult, op1=ALU.add)
                    if j >= 1:
                        jj = j - 1
                        cs2 = c_st[:, jj * 8:(jj + 1) * 8]
                        tnc = scr.tile([128, 8], F32, name=f"tnc{jj}",
                                       tag=f"tnc{jj}")
                        nc.scalar.activation(tnc[:], cs2, AF.Tanh, scale=0.5)
                        nc.vector.scalar_tensor_tensor(
                            hTn[:, jj * 8:(jj + 1) * 8], tact[jj][:, 16:24],
                            1.0, tnc[:], op0=ALU.add, op1=ALU.mult)

            oh0 = ohp.tile([V, 2 * CH * BLOC], DT, tag="oh")
            nc.sync.dma_start(out=oh0[:], in_=oh_d[:, 0:2 * CH * BLOC])
            produce_xg(oh0, 0, xgA)
            produce_xg(oh0, CH * BLOC, xgB)

            with tc.For_i(0, ITERS // 4, 1,
                          hint_engines=(mybir.EngineType.PE,)) as iv:
                for rep in range(4):
                    oh = ohp.tile([V, 2 * CH * BLOC], DT, tag="oh")
                    nc.sync.dma_start(
                        out=oh[:],
                        in_=oh_d[:, ds(iv * (8 * CH * BLOC)
                                       + (rep + 1) * (2 * CH * BLOC),
                                       2 * CH * BLOC)])
                    rbase = (iv * 4 + rep) * (BODY * 32)
                    for sc in range(CH):
                        a = hring[:, sc * 32:(sc + 1) * 32]
                        b = hring[:, (sc + 1) * 32:(sc + 2) * 32]
                        fil = (lambda m=sc: prod_mm(oh, 0, m))
                        step(sc, xgA, a, b, filler=fil)
                    nc.sync.dma_start(
                        out=hd_d[:, ds(rbase, CH * 32)],
                        in_=hring[:, 32:(CH + 1) * 32])
                    xg_copy(xgA)
                    for sc in range(CH):
                        a = hring[:, (CH + sc) * 32:(CH + sc + 1) * 32]
                        b = hring[:, (CH + sc + 1) * 32:(CH + sc + 2) * 32]
                        fil = (lambda m=sc: prod_mm(oh, CH * BLOC, m))
                        step(sc, xgB, a, b, filler=fil)
                    nc.sync.dma_start(
                        out=hd_d[:, ds(rbase + CH * 32, CH * 32)],
                        in_=hring[:, (CH + 1) * 32:(2 * CH + 1) * 32])
                    xg_copy(xgB)
                    nc.vector.tensor_copy(hring[:, 0:32],
                                          hring[:, 32 * CH * 2:32 * CH * 2 + 32])

    split_multi_waits(nc)
    return nc


def _gather_out(results, lens_sorted, fc_w, fc_b):
    fcw = np.asarray(fc_w, np.float32)[0]
    fcb = float(np.asarray(fc_b, np.float32)[0])
    out = np.zeros((N_CORES * BLOC, 1), np.float32)
    for ci in range(N_CORES):
        hd = results[ci]["hdump"]
        for b in range(BLOC):
            t = int(lens_sorted[ci * BLOC + b]) - 1
            h2 = np.concatenate(
                [hd[:, t * 32 + k * 8 + b].astype(np.float32)
                 for k in range(4)])
            out[ci * BLOC + b, 0] = 0.5 * float(np.dot(fcw, h2)) + fcb
    return out


_NC_CACHE = None


def kernel(tokens, lengths, W_ih, W_hh, b_ih, b_hh, fc_w, fc_b):
    global _NC_CACHE
    per_core, order = _host_prep(tokens, lengths, W_ih, W_hh, b_ih, b_hh,
                                 fc_w, fc_b)
    if _NC_CACHE is None:
        _NC_CACHE = _build_nc()
    res = run_bass_kernel_spmd(_NC_CACHE, per_core,
                               core_ids=list(range(N_CORES)))
    # reference returns outputs in sorted (desc length) order; shard ci
    # holds sorted ranks ci*8..ci*8+7, so this is already sorted order
    lens_sorted = np.asarray(lengths).astype(np.int64)[order]
    return _gather_out(res.results, lens_sorted, fc_w, fc_b)



# revision 7
# speedup vs baseline: 1.0015x; 1.0015x over previous
# Trainium2 Bass kernel for nn_BinaryClassifier (one-hot -> LSTM -> FC).
#
# Data-parallel over batch: 8 sorted sequences per NeuronCore, 8 cores run
# one program on different shards. Per core the LSTM runs 2048 sequential
# steps: each step streams W_hh through the PE as 64 bf16 [128,128]
# stationary tiles against the transposed h state ([128 hidden, 8 batch],
# kept in a 33-slot SBUF ring), with the embedding contribution
# (E = W_ih.T + biases, gathered by token) pre-accumulated into PSUM by an
# identity matmul; embedding columns are produced on the PE 16 steps at a
# time from host-built one-hot moving operands, interleaved into the step
# stream as stall filler. All gate nonlinearities use a single tanh per
# hidden slice (sigmoid(x) = (tanh(x/2)+1)/2 folded into pre-scaled
# weights; h stored as 2h, c as 2c). Whole chunks of h are DMAd to DRAM;
# the host gathers h at t = len-1 and applies the FC during unsharding.
import sys
sys.path.insert(0, '/opt/trn_rl_repo')
from contextlib import ExitStack

import numpy as np
import ml_dtypes

import concourse.bass as bass
import concourse.mybir as mybir
from concourse.tile import TileContext
from concourse.bass import ds
from concourse.bass_utils import run_bass_kernel_spmd

F32 = mybir.dt.float32
BF16 = mybir.dt.bfloat16
AF = mybir.ActivationFunctionType
ALU = mybir.AluOpType

H = 512
V = 25
S = 2048
N_CORES = 8
BLOC = 8          # sequences per core
CH = 16           # steps per embedding chunk
BODY = 2 * CH     # steps per chunk pair
NM = 16           # gate tiles (4H / 128)
NK = 4            # contraction tiles (H / 128)
W_SCALE = 4096.0  # lift fp8 W out of the subnormal range; gates carry the
                  # scale (xg pre-scaled on host) and tanh applies 2^-12

_TPB_ENGINES = None


def split_multi_waits(nc):
    """walrus in this container supports only ONE sync wait per TPB engine
    instruction; split extra waits onto preceding same-engine NOPs."""
    global _TPB_ENGINES
    if _TPB_ENGINES is None:
        _TPB_ENGINES = {mybir.EngineType.Pool, mybir.EngineType.Activation,
                        mybir.EngineType.PE, mybir.EngineType.DVE,
                        mybir.EngineType.SP}
    ctr = 0
    for fn in nc.m.functions:
        for bb in fn.blocks:
            new = []
            for inst in bb.instructions:
                si = inst.sync_info
                if (si is not None and len(si.on_wait) > 1
                        and inst.engine in _TPB_ENGINES):
                    waits = list(si.on_wait)
                    for w in waits[:-1]:
                        nop = mybir.InstNoOp(name=f"wsplit-{ctr}", ins=[],
                                             outs=[])
                        ctr += 1
                        nop.engine = inst.engine
                        nop.sync_info = mybir.SyncInfo(on_wait=[w],
                                                       on_update=[])
                        new.append(nop)
                    si.on_wait = waits[-1:]
                    inst.sync_info = si
                new.append(inst)
            bb.instructions = new


def _host_prep(tokens, lengths, W_ih, W_hh, b_ih, b_hh, fc_w, fc_b):
    """Full inputs -> list of per-core input dicts (numpy).

    Gate-tile numbering: m = j*4 + g where j = hidden slice (0..3) and
    g in {0:i, 1:f, 2:o, 3:g_cell} (reordered from torch i,f,g,o)."""
    bf = ml_dtypes.bfloat16
    order = np.argsort(-lengths.astype(np.int64), kind='stable')
    toks = np.asarray(tokens)[order]
    lens = np.asarray(lengths)[order].astype(np.int64)

    # rows of W_* are 4H in torch gate order i,f,g,o; our g order: i,f,o,g
    perm = np.concatenate([np.arange(0 * H, 1 * H),      # i
                           np.arange(1 * H, 2 * H),      # f
                           np.arange(3 * H, 4 * H),      # o
                           np.arange(2 * H, 3 * H)])     # g_cell
    # row index of gate-unit (g, j, c) in permuted matrix: g*H + j*128 + c
    Whh_p = np.asarray(W_hh)[perm].astype(np.float32)    # [4H, H]
    E_p = (np.asarray(W_ih) + np.asarray(b_ih)[:, None]
           + np.asarray(b_hh)[:, None])[perm].astype(np.float32)
    # sigmoid(x) = (tanh(x/2)+1)/2: pre-halve i,f,o gate rows so one tanh
    # covers all gates; h is stored as h2 = 2h, so W_hh is halved again.
    ifo = np.zeros(4 * H, bool)
    ifo[0:3 * H] = True                                   # i,f,o rows
    Whh_p[ifo] *= 0.5
    E_p[ifo] *= 0.5
    Whh_p *= 0.5                                          # h2 = 2h convention
    E_p *= W_SCALE

    # w_lhsT: [128, NK*NM*128], tile (k, m) at cols (k*NM+m)*128
    # m = j*4+g selects rows g*H + j*128 + (0..127); k selects hidden cols
    w = np.zeros((128, NK * NM * 128), np.float32)
    e = np.zeros((V, NM * 128), np.float32)
    for j in range(4):
        for g in range(4):
            m = j * 4 + g
            rows = slice(g * H + j * 128, g * H + j * 128 + 128)
            for k in range(NK):
                blk = Whh_p[rows, k * 128:(k + 1) * 128]   # [128 rows, 128 k]
                w[:, (k * NM + m) * 128:(k * NM + m + 1) * 128] = blk.T
            e[:, m * 128:(m + 1) * 128] = E_p[rows, :].T   # [V, 128]

    fcw = np.zeros((128, 4), np.float32)
    for j in range(4):
        # hfin holds h2 = 2h, fold the 0.5 into the FC weights
        fcw[:, j] = 0.5 * np.asarray(fc_w)[0, j * 128:(j + 1) * 128]

    f8 = ml_dtypes.float8_e4m3  # TRN fp8_e4m3 (max normal 240)
    per_core = []
    for ci in range(N_CORES):
        bs = slice(ci * BLOC, (ci + 1) * BLOC)
        t_c = toks[bs]                                    # [8, S]
        l_c = lens[bs]                                    # [8]
        oh = np.zeros((V, S * BLOC + 2 * CH * BLOC), np.float32)
        sidx = np.arange(S)
        for b in range(BLOC):
            oh[t_c[b], sidx * BLOC + b] = 1.0
        lcap = np.tile((l_c - 1).astype(np.float32), 4)   # col j*8+b
        lcap = np.broadcast_to(lcap, (128, 32)).copy()
        fcb = np.full((BLOC, 1), np.asarray(fc_b)[0], np.float32)
        per_core.append({
            "ident": np.eye(128, dtype=np.float32).astype(bf),
            "w_lhsT": (w * W_SCALE).astype(f8),
            "e_lhsT": e.astype(bf),
            "onehot": oh.astype(bf),
            "lcap": lcap,
            "fcw": fcw,
            "fcb": fcb,
        })
    return per_core, order


def _build_nc():
    assert S % BODY == 0
    ITERS = S // BODY
    nc = bass.Bass("TRN2", target_bir_lowering=False, debug=False,
                   num_devices=N_CORES)
    DT = BF16
    F8 = mybir.dt.float8e4
    w_d = nc.dram_tensor("w_lhsT", [128, NK * NM * 128], F8,
                         kind="ExternalInput").ap()
    e_d = nc.dram_tensor("e_lhsT", [V, NM * 128], DT,
                         kind="ExternalInput").ap()
    oh_d = nc.dram_tensor("onehot", [V, S * BLOC + 2 * CH * BLOC], DT,
                          kind="ExternalInput").ap()
    lcap_d = nc.dram_tensor("lcap", [128, 32], F32, kind="ExternalInput").ap()
    id_d = nc.dram_tensor("ident", [128, 128], DT, kind="ExternalInput").ap()
    fcw_d = nc.dram_tensor("fcw", [128, 4], F32, kind="ExternalInput").ap()
    fcb_d = nc.dram_tensor("fcb", [BLOC, 1], F32, kind="ExternalInput").ap()
    hd_d = nc.dram_tensor("hdump", [128, S * 32], BF16,
                          kind="ExternalOutput").ap()

    with TileContext(nc) as tc, ExitStack() as ctx:
        const = ctx.enter_context(tc.tile_pool(name="const", bufs=1))
        state = ctx.enter_context(tc.tile_pool(name="state", bufs=1))
        scr = ctx.enter_context(tc.tile_pool(name="scr", bufs=6))
        ohp = ctx.enter_context(tc.tile_pool(name="ohp", bufs=2))

        w_sb = const.tile([128, NK * NM * 128], F8, tag="w")
        e_sb = const.tile([V, NM * 128], DT, tag="e")
        lcap = const.tile([128, 32], F32, tag="lcap")
        fcw = const.tile([128, 4], F32, tag="fcw")
        fcb = const.tile([BLOC, 1], F32, tag="fcb")
        ident = const.tile([128, 128], DT, tag="ident")
        nc.sync.dma_start(out=w_sb[:], in_=w_d[:])
        nc.sync.dma_start(out=e_sb[:], in_=e_d[:])
        nc.sync.dma_start(out=lcap[:], in_=lcap_d[:])
        nc.sync.dma_start(out=fcw[:], in_=fcw_d[:])
        nc.sync.dma_start(out=fcb[:], in_=fcb_d[:])
        nc.sync.dma_start(out=ident[:], in_=id_d[:])

        # h ring: body step i reads slot i, writes slot i+1 (33 slots);
        # slot 32 is copied back to slot 0 at body end. Whole chunks of h
        # are DMAd to DRAM so the host can gather h at t = len-1.
        hring = state.tile([128, 33 * 32], DT, tag="hring")
        c_st = state.tile([128, 32], F32, tag="c")
        nc.vector.memset(hring[:, 0:32], 0)
        nc.vector.memset(c_st[:], 0)

        # xg buffers: col sc*128 + m*8 + b  (t-major after permuted copy)
        xgA = state.tile([128, CH * 128], DT, tag="xgA")
        xgB = state.tile([128, CH * 128], DT, tag="xgB")

        with tc.tile_pool(name="psum", bufs=1, space="PSUM") as psum:
            gp = [psum.tile([128, 32], F32, name=f"gp{j}", tag=f"gp{j}")
                  for j in range(4)]
            xgp = psum.tile([128, CH * 128], F32, tag="xgp")

            def prod_mm(oh_tile, col0, m):
                nc.tensor.matmul(
                    xgp[:, m * CH * BLOC:(m + 1) * CH * BLOC],
                    e_sb[:, m * 128:(m + 1) * 128],
                    oh_tile[:, col0:col0 + CH * BLOC],
                    start=(m % 4 == 0), stop=(m % 4 == 3))

            def xg_copy(xg_sb):
                src = xgp[:].rearrange("p (m t b) -> p t m b",
                                       m=NM, t=CH, b=BLOC)
                dst = xg_sb[:].rearrange("p (t m b) -> p t m b",
                                         m=NM, t=CH, b=BLOC)
                nc.scalar.copy(dst, src)

            def produce_xg(oh_tile, col0, xg_sb):
                for m in range(NM):
                    prod_mm(oh_tile, col0, m)
                xg_copy(xg_sb)

            def step(sc, xg_sb, hT, hTn, filler=None):
                # adjacent identity injections (slices 0-2) so ldw-opt can
                # elide the repeated stationary loads; slice 3's stays at
                # its block to avoid a WAR stall on the prior step's tanh
                for j in range(3):
                    nc.tensor.matmul(
                        gp[j][:], ident[:],
                        xg_sb[:, sc * 128 + j * 32:sc * 128 + (j + 1) * 32],
                        start=True, stop=False)
                for j in range(4):
                    if j == 3:
                        nc.tensor.matmul(
                            gp[3][:], ident[:],
                            xg_sb[:, sc * 128 + 96:sc * 128 + 128],
                            start=True, stop=False)
                        if filler is not None:
                            filler()
                    for k in range(NK):
                        for g in range(4):
                            m = j * 4 + g
                            nc.tensor.matmul(
                                gp[j][:, g * 8:(g + 1) * 8],
                                w_sb[:, (k * NM + m) * 128:
                                     (k * NM + m + 1) * 128],
                                hT[:, k * 8:(k + 1) * 8],
                                start=False,
                                stop=(k == NK - 1 and g == 3))
                # two-phase staggered tails: emit slice j's tanh_c/h2 after
                # slice j+1's tanh_all so the last slice's tanh is not
                # queued behind the previous slice's chain on ACT
                tact = [None] * 4
                for j in range(5):
                    if j < 4:
                        tact[j] = scr.tile([128, 32], F32, name=f"ta{j}",
                                           tag=f"tact{j}")
                        nc.scalar.activation(tact[j][:], gp[j][:], AF.Tanh,
                                             scale=1.0 / W_SCALE)
                        cs = c_st[:, j * 8:(j + 1) * 8]
                        t1 = scr.tile([128, 8], F32, name=f"t1_{j}",
                                      tag=f"t1_{j}")
                        t2 = scr.tile([128, 8], F32, name=f"t2_{j}",
                                      tag=f"t2_{j}")
                        nc.vector.scalar_tensor_tensor(
                            t1[:], tact[j][:, 0:8], 1.0, tact[j][:, 24:32],
                            op0=ALU.add, op1=ALU.mult)
                        nc.vector.scalar_tensor_tensor(
                            t2[:], tact[j][:, 8:16], 1.0, cs,
                            op0=ALU.add, op1=ALU.mult)
                        nc.vector.scalar_tensor_tensor(
                            cs, t2[:], 0.5, t1[:], op0=ALU.mult, op1=ALU.add)
                    if j >= 1:
                        jj = j - 1
                        cs2 = c_st[:, jj * 8:(jj + 1) * 8]
                        tnc = scr.tile([128, 8], F32, name=f"tnc{jj}",
                                       tag=f"tnc{jj}")
                        nc.scalar.activation(tnc[:], cs2, AF.Tanh, scale=0.5)
                        nc.vector.scalar_tensor_tensor(
                            hTn[:, jj * 8:(jj + 1) * 8], tact[jj][:, 16:24],
                            1.0, tnc[:], op0=ALU.add, op1=ALU.mult)

            oh0 = ohp.tile([V, 2 * CH * BLOC], DT, tag="oh")
            nc.sync.dma_start(out=oh0[:], in_=oh_d[:, 0:2 * CH * BLOC])
            produce_xg(oh0, 0, xgA)
            produce_xg(oh0, CH * BLOC, xgB)

            with tc.For_i(0, ITERS // 4, 1,
                          hint_engines=(mybir.EngineType.PE,)) as iv:
                for rep in range(4):
                    oh = ohp.tile([V, 2 * CH * BLOC], DT, tag="oh")
                    nc.sync.dma_start(
                        out=oh[:],
                        in_=oh_d[:, ds(iv * (8 * CH * BLOC)
                                       + (rep + 1) * (2 * CH * BLOC),
                                       2 * CH * BLOC)])
                    rbase = (iv * 4 + rep) * (BODY * 32)
                    for sc in range(CH):
                        a = hring[:, sc * 32:(sc + 1) * 32]
                        b = hring[:, (sc + 1) * 32:(sc + 2) * 32]
                        fil = (lambda m=sc: prod_mm(oh, 0, m))
                        step(sc, xgA, a, b, filler=fil)
                    nc.sync.dma_start(
                        out=hd_d[:, ds(rbase, CH * 32)],
                        in_=hring[:, 32:(CH + 1) * 32])
                    xg_copy(xgA)
                    for sc in range(CH):
                        a = hring[:, (CH + sc) * 32:(CH + sc + 1) * 32]
                        b = hring[:, (CH + sc + 1) * 32:(CH + sc + 2) * 32]
                        fil = (lambda m=sc: prod_mm(oh, CH * BLOC, m))
                        step(sc, xgB, a, b, filler=fil)
                    nc.sync.dma_start(
                        out=hd_d[:, ds(rbase + CH * 32, CH * 32)],
                        in_=hring[:, (CH + 1) * 32:(2 * CH + 1) * 32])
                    xg_copy(xgB)
                    nc.vector.tensor_copy(hring[:, 0:32],
                                          hring[:, 32 * CH * 2:32 * CH * 2 + 32])

    split_multi_waits(nc)
    return nc


def _gather_out(results, lens_sorted, fc_w, fc_b):
    fcw = np.asarray(fc_w, np.float32)[0]
    fcb = float(np.asarray(fc_b, np.float32)[0])
    out = np.zeros((N_CORES * BLOC, 1), np.float32)
    for ci in range(N_CORES):
        hd = results[ci]["hdump"]
        for b in range(BLOC):
            t = int(lens_sorted[ci * BLOC + b]) - 1
            h2 = np.concatenate(
                [hd[:, t * 32 + k * 8 + b].astype(np.float32)
                 for k in range(4)])
            out[ci * BLOC + b, 0] = 0.5 * float(np.dot(fcw, h2)) + fcb
    return out


_NC_CACHE = None


def kernel(tokens, lengths, W_ih, W_hh, b_ih, b_hh, fc_w, fc_b):
    global _NC_CACHE
    per_core, order = _host_prep(tokens, lengths, W_ih, W_hh, b_ih, b_hh,
                                 fc_w, fc_b)
    if _NC_CACHE is None:
        _NC_CACHE = _build_nc()
    res = run_bass_kernel_spmd(_NC_CACHE, per_core,
                               core_ids=list(range(N_CORES)))
    # reference returns outputs in sorted (desc length) order; shard ci
    # holds sorted ranks ci*8..ci*8+7, so this is already sorted order
    lens_sorted = np.asarray(lengths).astype(np.int64)[order]
    return _gather_out(res.results, lens_sorted, fc_w, fc_b)

